# revision 6
# baseline (speedup 1.0000x reference)
"""Trainium2 Bass kernel for LoRALayer: out = 2.0 * (x @ B) @ A.

x: [4, 4096, 4096] f32; A: [8, 4096] f32; B: [4096, 8] f32.
Sharding: data-parallel on the 16384 tokens across 8 cores (2048 each);
A/B replicated. Host-side prep (part of sharding): each core's x-shard is
shipped transposed (xT [4096, 2048]) so the contraction dim lies on SBUF
partitions; B is pre-packed chunk-major; A is pre-scaled by 2.

Per core (all f32, exact):
  mm1 (PE): yT[8, 512] += Bp_c[128, 8].T @ xT_c[128, 512]   (32 chunks)
  mm2 (PE): out[128, 512] = yT_sub[8, 128].T @ A2[8, 512]    (per 128-tok, 8 n-chunks)
PSUM->SBUF copies alternate DVE/ACT; HWDGE DMAs.
"""

import numpy as np

P = 128
F_IN = 4096
F_OUT = 4096
RANK = 8
N_CORES = 8
SCALING = 2.0
TBLK = 512             # token block (mm1 rhs free dim, f32 max 512)

_CACHE = {}


def _build_nc(T, F_in, F_out, R):
    """Build the single-core Bass program for a T-token shard."""
    from contextlib import ExitStack

    import concourse.mybir as mybir
    import concourse.tile as tile
    from concourse import bacc

    f32 = mybir.dt.float32
    tblk = min(TBLK, T)     # token block (mm1 rhs free dim, f32 max 512)
    CH = F_in // P          # feature chunks (32)
    NB = T // tblk          # token blocks (4)
    NSUB = tblk // P        # 128-token subtiles per block (4)
    NS = F_out // 512       # output column chunks (8)
    CGRP = min(8, CH)       # chunks per input sub-DMA

    nc = bacc.Bacc("TRN2", target_bir_lowering=False, debug=False)

    xt_d = nc.dram_tensor("xT", [F_in, T], f32, kind="ExternalInput").ap()
    bp_d = nc.dram_tensor("Bp", [P, CH * R], f32, kind="ExternalInput").ap()
    a2_d = nc.dram_tensor("A2", [R, F_out], f32, kind="ExternalInput").ap()
    out_d = nc.dram_tensor("out", [T, F_out], f32, kind="ExternalOutput").ap()

    with tile.TileContext(nc) as tc, ExitStack() as ctx:
        cpool = ctx.enter_context(tc.tile_pool(name="const", bufs=1))
        xtpool = ctx.enter_context(tc.tile_pool(name="xt", bufs=8))
        ytpool = ctx.enter_context(tc.tile_pool(name="yt", bufs=3))
        opool = ctx.enter_context(tc.tile_pool(name="osb", bufs=3))
        y_pp = ctx.enter_context(tc.tile_pool(name="y_ps", bufs=2, space="PSUM"))
        o_pp = ctx.enter_context(tc.tile_pool(name="o_ps", bufs=4, space="PSUM"))

        b_sb = cpool.tile([P, CH * R], f32, tag="b_sb")
        nc.sync.dma_start(b_sb[:], bp_d)
        a_sb = cpool.tile([R, F_out], f32, tag="a_sb")
        nc.sync.dma_start(a_sb[:], a2_d)

        for blk in range(NB):
            t0 = blk * tblk
            # load xT[:, t0:t0+TBLK] as 4 sub-DMAs of CGRP chunks each:
            # tile [128, CGRP, TBLK], partition p row c holds xT[128c+p, t0:t0+TBLK]
            xts = []
            src = xt_d[:, t0:t0 + tblk].rearrange("(c p) t -> p c t", p=P)
            for s in range(CH // CGRP):
                xt_sb = xtpool.tile([P, CGRP, tblk], f32, tag="xt_sb")
                nc.sync.dma_start(xt_sb[:], src[:, s * CGRP:(s + 1) * CGRP, :])
                xts.append(xt_sb)

            yt_ps = y_pp.tile([R, tblk], f32, tag="yt_ps")
            for c in range(CH):
                nc.tensor.matmul(
                    yt_ps[:],
                    b_sb[:, c * R:(c + 1) * R],
                    xts[c // CGRP][:, c % CGRP, :],
                    start=(c == 0),
                    stop=(c == CH - 1),
                )
            yt_sb = ytpool.tile([R, tblk], f32, tag="yt_sb")
            nc.vector.tensor_copy(yt_sb[:], yt_ps[:])

            for sub in range(NSUB):
                trow = t0 + sub * P
                o_sb = opool.tile([P, F_out], f32, tag="o_sb")
                for n in range(NS):
                    o_ps = o_pp.tile([P, 512], f32, tag="o_ps")
                    nc.tensor.matmul(
                        o_ps[:],
                        yt_sb[:, sub * P:(sub + 1) * P],
                        a_sb[:, n * 512:(n + 1) * 512],
                        start=True,
                        stop=True,
                    )
                    if n % 2 == 0:
                        nc.scalar.copy(o_sb[:, n * 512:(n + 1) * 512], o_ps[:])
                    else:
                        nc.vector.tensor_copy(o_sb[:, n * 512:(n + 1) * 512], o_ps[:])
                nc.sync.dma_start(out_d[trow:trow + P, :], o_sb[:])

    nc.compile()
    return nc


def _pack_inputs(x2d, A, B, T_shard, F_in, R):
    """Shard x on tokens (shipped transposed); replicate packed B and 2*A."""
    CH = F_in // P
    bp = np.ascontiguousarray(
        B.reshape(CH, P, R).transpose(1, 0, 2).reshape(P, CH * R)
    ).astype(np.float32)
    a2 = np.ascontiguousarray(SCALING * A).astype(np.float32)
    n_shards = x2d.shape[0] // T_shard
    in_maps = []
    for c in range(n_shards):
        xt = np.ascontiguousarray(x2d[c * T_shard:(c + 1) * T_shard].T)
        in_maps.append({"xT": xt, "Bp": bp, "A2": a2})
    return in_maps


def kernel(x, A, B):
    from concourse.bass_utils import run_bass_kernel_spmd

    x = np.asarray(x, dtype=np.float32)
    A = np.asarray(A, dtype=np.float32)
    B = np.asarray(B, dtype=np.float32)
    orig_shape = x.shape
    x2d = x.reshape(-1, F_IN)
    T_shard = x2d.shape[0] // N_CORES

    key = (T_shard, F_IN, F_OUT, RANK)
    if key not in _CACHE:
        _CACHE[key] = _build_nc(T_shard, F_IN, F_OUT, RANK)
    nc = _CACHE[key]

    in_maps = _pack_inputs(x2d, A, B, T_shard, F_IN, RANK)
    res = run_bass_kernel_spmd(nc, in_maps, core_ids=list(range(N_CORES)))
    out = np.concatenate([r["out"] for r in res.results], axis=0)
    return out.reshape(*orig_shape[:-1], F_OUT)
